# revision 17
# baseline (speedup 1.0000x reference)
"""Trainium2 Bass kernel for nn_Attention_73718818669284.

Reference computation (per batch b of 2, C=128 channels, N=4096 spatial):
    q = Wq x, k = Wk x, v = Wv x           (1x1 conv == channel matmul)
    w = softmax(q^T k, axis=-1)            ([N, N] attention)
    h = Wo (v w^T)
    y = x + h
    out = SiLU(GroupNorm8(y) * gamma + beta)

Sharding: 8 cores = 2 batches x 4 column-slices of N (1024 each).
Each core computes its slice of the attention output; GroupNorm statistics
are combined across the 4 cores of a batch with a tiny AllReduce (a
warm-up collective at kernel entry wakes the CC cores and absorbs the
cross-core start stagger in parallel with the prologue DMAs).

Per-core algorithm (transposed-score layout -> no PE transposes of P):
    M   = Wq^T Wk                     (one 128x128 matmul, fp16)
    R   = M^T X_s                     ([128, 1024] fp16, folds q-projection)
    S^T chunk j = X[:,128j:]^T R      ([128m, 1024n]; scores, fp16 inputs)
    P^T = exp(S^T + shift)            (bf16; shift cancels in softmax)
    rowsum = sum_m P^T[m, n]          (DVE bf16 dual accumulators)
    h_un = V P = sum_j VT_j^T PT_j    (V^T via PE transpose mode, bf16)
    h = h_un * (1/rowsum)             (1/r = exp(-ln r) on ACT, set 6)
    y = Wo h + x_s ; stats exchange; GroupNorm; SiLU.

Matmul dtypes: the score path (X, M, R, Wo/h) runs in fp16 (10-bit
mantissa keeps score errors ~1e-3; bf16 scores measure 3e-2 rel err),
the P-side (exp output, V, rowsums) in bf16 (fp16 would overflow at
e^41). Both stream 1 cycle/row with fast weight load; the fp32 moving
path measures ~2x slower. The ACT table set 6 (exp+ln+square) is pinned
at kernel entry; the only switch (silu) hides under the stats exchange.
GroupNorm rstd uses the DVE rsqrt bit-trick + 2 Newton steps.
"""

import numpy as np

import concourse.bass as bass
import concourse.tile as tile
from concourse import bacc, mybir
from concourse.bass_utils import run_bass_kernel_spmd

F32 = mybir.dt.float32
F16 = mybir.dt.float16
BF16 = mybir.dt.bfloat16
NPBF16 = mybir.dt.np(mybir.dt.bfloat16)
AF = mybir.ActivationFunctionType
ALU = mybir.AluOpType
AX = mybir.AxisListType

P = 128          # channels / partitions
N = 4096         # spatial size (16*16*16)
NS = 1024        # per-core slice of N
NB = N // P      # 32 m-chunks
NCORES = 8
NGROUPS = 8
EPS = 1e-5
CNT = (P // NGROUPS) * N   # elements per group per batch = 16 * 4096
NPB = 4 * P                # fp16 params width (wq|wk|wvT|woT)
NPF = NGROUPS + 4          # fp32 params width (gsel/CNT | gamma | beta | bsel0 | bsel1)
ACT_SET_MAIN = 6           # natural_log_exp_and_others: exp + ln + square


def _load_act_set(nc, set_id):
    return nc.scalar.add_instruction(
        mybir.InstLoadActFuncSet(
            name=nc.get_next_instruction_name(),
            ins=[], outs=[],
            act_func_set_id=set_id,
        )
    )


def _build_nc():
    nc = bacc.Bacc("TRN2", target_bir_lowering=False, debug=False,
                   num_devices=NCORES)

    xb = nc.declare_dram_parameter("xb", [P, N], F16, isOutput=False)
    xs16 = nc.declare_dram_parameter("xs16", [P, NS], F16, isOutput=False)
    xs = nc.declare_dram_parameter("xs", [P, NS], F32, isOutput=False)
    pb = nc.declare_dram_parameter("pb", [P, NPB], F16, isOutput=False)
    idb = nc.declare_dram_parameter("idb", [P, P], BF16, isOutput=False)
    pf = nc.declare_dram_parameter("pf", [P, NPF], F32, isOutput=False)
    gselT = nc.declare_dram_parameter("gselT", [NGROUPS, P], F32,
                                      isOutput=False)
    out = nc.declare_dram_parameter("out", [P, NS], F32, isOutput=True)

    with tile.TileContext(nc) as tc:
        _emit(nc, tc, xb, xs16, xs, pb, idb, pf, gselT, out)
    nc.compile()
    return nc


def _emit(nc, tc, xb, xs16, xs, pb, idb, pf, gselT, out):
    with (
        tc.tile_pool(name="pp", bufs=1) as pp,
        tc.tile_pool(name="ptp", bufs=4) as ptp,
        tc.tile_pool(name="dp", bufs=1, space="DRAM") as dp,
    ):
        # Pin the exp+ln+square table set before any ACT op; every
        # loop/epilogue activation is then satisfied and the only
        # remaining switch (silu) hides under the stats exchange.
        _load_act_set(nc, ACT_SET_MAIN)

        # ---------------- loads (two HWDGE rings in parallel) -----------
        pb_sb = pp.tile([P, NPB], F16)
        nc.scalar.dma_start(out=pb_sb[:], in_=pb[:])
        xsr = pp.tile([P, NS], F16)
        nc.scalar.dma_start(out=xsr[:], in_=xs16[:])
        id_sb = pp.tile([P, P], BF16)
        nc.scalar.dma_start(out=id_sb[:], in_=idb[:])
        pf_sb = pp.tile([P, NPF], F32)
        nc.scalar.dma_start(out=pf_sb[:], in_=pf[:])
        gselT_sb = pp.tile([NGROUPS, P], F32)
        nc.scalar.dma_start(out=gselT_sb[:], in_=gselT[:])
        # warm-up collective: aligns core start (prevents a fast core's
        # remote stats write racing a slow core's semaphore clear) and
        # wakes the CC cores in parallel with the prologue DMAs
        warm = pp.tile([1, 2], F32)
        nc.vector.memset(warm[:], 0.0)
        dumc_in = dp.tile([1, 2], F32)
        dumc_out = dp.tile([1, 2], F32)
        nc.sync.dma_start(out=dumc_in[:], in_=warm[:])
        nc.gpsimd.collective_compute(
            "AllReduce", ALU.add,
            replica_groups=[[0, 1, 2, 3], [4, 5, 6, 7]],
            ins=[dumc_in.opt()], outs=[dumc_out.opt()],
        )
        xs_sb = pp.tile([P, NS], F32)
        nc.scalar.dma_start(out=xs_sb[:], in_=xs[:])
        xr = pp.tile([P, N], F16)
        for i in range(4):
            nc.sync.dma_start(out=xr[:, i * NS:(i + 1) * NS],
                              in_=xb[:, i * NS:(i + 1) * NS])
        wq_b = pb_sb[:, 0:128]
        wk_b = pb_sb[:, 128:256]
        wvT_b = pb_sb[:, 256:384]
        woT_b = pb_sb[:, 384:512]
        gsel_c = pf_sb[:, 0:NGROUPS]        # scaled by 1/CNT host-side
        gamma_sb = pf_sb[:, NGROUPS:NGROUPS + 1]
        beta_sb = pf_sb[:, NGROUPS + 1:NGROUPS + 2]
        bsel0 = pf_sb[:, NGROUPS + 2:NGROUPS + 3]   # 1.0 iff batch-0 core
        bsel1 = pf_sb[:, NGROUPS + 3:NGROUPS + 4]   # 1.0 iff batch-1 core

        gselT_c = pp.tile([NGROUPS, P], F32)
        nc.vector.tensor_copy(gselT_c[:], gselT_sb[:])
        onesM = pp.tile([P, P], BF16)
        nc.vector.memset(onesM[:], 1.0)
        # Global exp shift: cancels exactly in softmax. Centers the
        # log-rowsum range [21.6, 103.5] inside exp/ln's clean window.
        shift = pp.tile([P, 1], F32)
        nc.vector.memset(shift[:], -62.5)

        stat_sb = pp.tile([P, 2], F32)

        # ------------- projections + attention loop (interleaved) -------
        r_r = pp.tile([P, NS], F16)
        v_sb = pp.tile([P, N], BF16)
        vt_sb = pp.tile([P, NB, P], BF16)
        h_sb = pp.tile([P, NS], F16)
        rsA = pp.tile([P, NS], BF16)
        rsB = pp.tile([P, NS], BF16)
        with (
            tc.tile_pool(name="stp", bufs=2, space="PSUM") as stp,
            tc.tile_pool(name="acc", bufs=1, space="PSUM") as acc,
        ):
            h_ps = acc.tile([P, NS], F32, tag="h")

            # M = Wq^T Wk  -> R = M^T Xs
            at_ps = stp.tile([P, P], F32, tag="st", name="at_ps")
            nc.tensor.matmul(at_ps[:], wq_b, wk_b, start=True, stop=True)
            at_b = pp.tile([P, P], F16)
            nc.vector.tensor_copy(at_b[:], at_ps[:])
            r_ps = stp.tile([P, NS], F32, tag="st", name="r_ps")
            nc.tensor.matmul(r_ps[:, 0:512], at_b[:], xsr[:, 0:512],
                             start=True, stop=True)
            nc.tensor.matmul(r_ps[:, 512:NS], at_b[:], xsr[:, 512:NS],
                             start=True, stop=True)
            nc.vector.tensor_copy(r_r[:, 0:512], r_ps[:, 0:512])
            nc.vector.tensor_copy(r_r[:, 512:NS], r_ps[:, 512:NS])

            def emit_vgroup(g):
                # V chunk g = Wv X[:, 512g:512g+512], then 4 PE transposes
                v_ps = stp.tile([P, 512], F32, tag="v", bufs=1,
                                name=f"v_ps{g}")
                nc.tensor.matmul(v_ps[:], wvT_b,
                                 xr[:, 512 * g:512 * (g + 1)],
                                 start=True, stop=True)
                nc.vector.tensor_copy(v_sb[:, 512 * g:512 * (g + 1)], v_ps[:])
                vt_ps = stp.tile([P, 4, P], BF16, tag="vt", bufs=1,
                                 name=f"vt_ps{g}")
                for t in range(4):
                    jj = 4 * g + t
                    nc.tensor.transpose(vt_ps[:, t, :],
                                        v_sb[:, jj * P:(jj + 1) * P], id_sb[:])
                nc.vector.tensor_copy(vt_sb[:, 4 * g:4 * g + 4, :], vt_ps[:])

            def consume(jj, ptj):
                first = jj == 0
                last = jj == NB - 1
                nc.tensor.matmul(h_ps[:, 0:512], vt_sb[:, jj, :], ptj[:, 0:512],
                                 start=first, stop=last)
                nc.tensor.matmul(h_ps[:, 512:NS], vt_sb[:, jj, :], ptj[:, 512:NS],
                                 start=first, stop=last)

            def rs_add(jj, ptj):
                # dual bf16 accumulators: 2x DVE mode, halved error depth
                dst = rsA if jj % 2 == 0 else rsB
                if jj < 2:
                    nc.vector.tensor_copy(dst[:], ptj[:])
                else:
                    nc.vector.tensor_add(dst[:], dst[:], ptj[:])

            # scores start immediately (need only xr chunk 0 + R); V/V^T
            # groups are woven into every 4th early iteration; PV matmuls lag
            # two iterations, the DVE row-sum adds lag one.
            vg_at = {2 + 4 * g: g for g in range(8)}   # j -> group
            pts = []
            for j in range(NB):
                if j in vg_at:
                    emit_vgroup(vg_at[j])
                st_ps = stp.tile([P, NS], F32, tag="st", name=f"st_ps{j}")
                lhs = xr[:, j * P:(j + 1) * P]
                nc.tensor.matmul(st_ps[:, 0:512], lhs, r_r[:, 0:512],
                                 start=True, stop=True)
                nc.tensor.matmul(st_ps[:, 512:NS], lhs, r_r[:, 512:NS],
                                 start=True, stop=True)
                pt = ptp.tile([P, NS], BF16, tag="pt", name=f"pt{j}")
                nc.scalar.activation(pt[:], st_ps[:], AF.Exp, bias=shift[:])
                pts.append(pt)
                if j >= 2:
                    consume(j - 2, pts[j - 2])
                if j >= 1:
                    rs_add(j - 1, pts[j - 1])
            for jj in (NB - 2, NB - 1):
                consume(jj, pts[jj])
            rs_add(NB - 1, pts[NB - 1])

            # broadcast-fold both accumulators with an all-ones stationary,
            # summing them in PSUM: rb[p, n] = rowsum[n] on every partition.
            rb_ps = stp.tile([P, NS], F32, tag="st", name="rb_ps")
            nc.tensor.matmul(rb_ps[:, 0:512], onesM[:], rsA[:, 0:512],
                             start=True, stop=False)
            nc.tensor.matmul(rb_ps[:, 0:512], onesM[:], rsB[:, 0:512],
                             start=False, stop=True)
            nc.tensor.matmul(rb_ps[:, 512:NS], onesM[:], rsA[:, 512:NS],
                             start=True, stop=False)
            nc.tensor.matmul(rb_ps[:, 512:NS], onesM[:], rsB[:, 512:NS],
                             start=False, stop=True)

            # 1/rowsum = exp(-ln(rowsum)): both in the pinned table set;
            # covers the whole fp32 range unlike the ACT reciprocal.
            lnr = pp.tile([P, NS], F32)
            nc.scalar.activation(lnr[:], rb_ps[:], AF.Ln)
            rbinv = pp.tile([P, NS], F32)
            nc.scalar.activation(rbinv[:], lnr[:], AF.Exp, scale=-1.0)

            # h = h_un / rowsum (fp16 for the Wo matmul), by halves so the
            # Wo matmul overlaps the second multiply
            nc.vector.tensor_mul(h_sb[:, 0:512], h_ps[:, 0:512],
                                 rbinv[:, 0:512])
            nc.vector.tensor_mul(h_sb[:, 512:NS], h_ps[:, 512:NS],
                                 rbinv[:, 512:NS])

        # ------------- output projection + residual + GroupNorm + SiLU ----
        with tc.tile_pool(name="ep", bufs=1, space="PSUM") as ep:
            a_ps = ep.tile([P, NS], F32, tag="a")
            nc.tensor.matmul(a_ps[:, 0:512], woT_b, h_sb[:, 0:512],
                             start=True, stop=True)
            nc.tensor.matmul(a_ps[:, 512:NS], woT_b, h_sb[:, 512:NS],
                             start=True, stop=True)
            y_sb = pp.tile([P, NS], F32)
            nc.vector.tensor_add(y_sb[:, 0:512], a_ps[:, 0:512],
                                 xs_sb[:, 0:512])
            nc.vector.tensor_add(y_sb[:, 512:NS], a_ps[:, 512:NS],
                                 xs_sb[:, 512:NS])

            # per-channel partial stats over the local 1024 columns; halves
            # so the first half's reductions overlap the second half's add
            hsum = pp.tile([P, 2], F32)
            nc.vector.reduce_sum(hsum[:, 0:1], y_sb[:, 0:512], axis=AX.X)
            sq_sb = pp.tile([P, NS], F32)
            nc.scalar.activation(sq_sb[:, 0:512], y_sb[:, 0:512], AF.Square,
                                 accum_out=hsum[:, 1:2])
            hsum2 = pp.tile([P, 2], F32)
            nc.vector.reduce_sum(hsum2[:, 0:1], y_sb[:, 512:NS], axis=AX.X)
            nc.scalar.activation(sq_sb[:, 512:NS], y_sb[:, 512:NS], AF.Square,
                                 accum_out=hsum2[:, 1:2])
            nc.vector.tensor_add(stat_sb[:], hsum[:], hsum2[:])

            # AllReduce within each batch's 4 cores; preload the silu
            # table set while the collective is in flight
            d_st1 = dp.tile([P, 2], F32)
            d_st2 = dp.tile([P, 2], F32)
            nc.sync.dma_start(out=d_st1[:], in_=stat_sb[:])
            dumo = pp.tile([1, 1], F32)
            nc.scalar.activation(dumo[:], stat_sb[0:1, 0:1], AF.Silu)
            nc.gpsimd.collective_compute(
                "AllReduce", ALU.add,
                replica_groups=[[0, 1, 2, 3], [4, 5, 6, 7]],
                ins=[d_st1.opt()], outs=[d_st2.opt()],
            )
            ast_sb = pp.tile([P, 2], F32)
            nc.sync.dma_start(out=ast_sb[:], in_=d_st2[:])
            ast_c = pp.tile([P, 2], F32)
            nc.vector.tensor_copy(ast_c[:], ast_sb[:])

            # fold channels -> groups; gsel carries 1/CNT so this yields
            # [mean, E[y^2]] per group directly
            gs_ps = ep.tile([NGROUPS, 2], F32, tag="gs")
            nc.tensor.matmul(gs_ps[:], gsel_c, ast_c[:], start=True, stop=True)
            mg = pp.tile([NGROUPS, 2], F32)
            nc.vector.tensor_copy(mg[:], gs_ps[:])
            msq = pp.tile([NGROUPS, 1], F32)
            nc.vector.tensor_mul(msq[:], mg[:, 0:1], mg[:, 0:1])
            var8 = pp.tile([NGROUPS, 1], F32)
            nc.vector.tensor_sub(var8[:], mg[:, 1:2], msq[:])
            # rstd = 1/sqrt(var + eps) via bit-trick + 2 Newton steps on the
            # DVE ([8,1] tiles) — keeps the pinned ACT table set intact.
            ve8 = pp.tile([NGROUPS, 1], F32)
            nc.vector.tensor_scalar_add(ve8[:], in0=var8[:], scalar1=EPS)
            I32 = mybir.dt.int32
            magic = pp.tile([NGROUPS, 1], I32)
            nc.vector.memset(magic[:], 0x5F3759DF)
            ish = pp.tile([NGROUPS, 1], I32)
            nc.vector.tensor_scalar(out=ish[:], in0=ve8.bitcast(I32),
                                    scalar1=1, scalar2=None,
                                    op0=ALU.arith_shift_right)
            y0i = pp.tile([NGROUPS, 1], I32)
            nc.vector.tensor_sub(y0i[:], magic[:], ish[:])
            ycur = y0i.bitcast(F32)
            for it in range(2):
                yy = pp.tile([NGROUPS, 1], F32, name=f"yy{it}")
                nc.vector.tensor_mul(yy[:], ycur[:], ycur[:])
                vy2 = pp.tile([NGROUPS, 1], F32, name=f"vy2{it}")
                nc.vector.tensor_mul(vy2[:], ve8[:], yy[:])
                hh = pp.tile([NGROUPS, 1], F32, name=f"hh{it}")
                nc.vector.tensor_scalar(out=hh[:], in0=vy2[:], scalar1=-0.5,
                                        scalar2=1.5, op0=ALU.mult, op1=ALU.add)
                ynew = pp.tile([NGROUPS, 1], F32, name=f"ynew{it}")
                nc.vector.tensor_mul(ynew[:], ycur[:], hh[:])
                ycur = ynew
            gval = pp.tile([NGROUPS, 2], F32)
            nc.vector.tensor_copy(gval[:, 0:1], mg[:, 0:1])
            nc.vector.tensor_copy(gval[:, 1:2], ycur[:])

            # broadcast group stats back to channels: [128, 2] = G @ gval
            pc_ps = ep.tile([P, 2], F32, tag="pc")
            nc.tensor.matmul(pc_ps[:], gselT_c[:], gval[:], start=True, stop=True)
            pc_sb = pp.tile([P, 2], F32)
            nc.vector.tensor_copy(pc_sb[:], pc_ps[:])

            # fuse (y - mean)*rstd*gamma + beta into one pass:
            # A = rstd*gamma, B = beta - mean*A, z = y*A + B
            A_sb = pp.tile([P, 1], F32)
            nc.vector.tensor_mul(A_sb[:], pc_sb[:, 1:2], gamma_sb)
            t_sb = pp.tile([P, 1], F32)
            nc.vector.tensor_mul(t_sb[:], pc_sb[:, 0:1], A_sb[:])
            B_sb = pp.tile([P, 1], F32)
            nc.vector.tensor_sub(B_sb[:], beta_sb, t_sb[:])
            z_sb = pp.tile([P, NS], F32)
            nc.vector.tensor_scalar(out=z_sb[:], in0=y_sb[:],
                                    scalar1=A_sb[:], scalar2=B_sb[:],
                                    op0=ALU.mult, op1=ALU.add)
            o_sb = pp.tile([P, NS], F32)
            nc.scalar.activation(o_sb[:], z_sb[:], AF.Silu)
            nc.sync.dma_start(out=out[:], in_=o_sb[:])


_NC_CACHE = None


def _get_nc():
    global _NC_CACHE
    if _NC_CACHE is None:
        _NC_CACHE = _build_nc()
    return _NC_CACHE


def make_in_maps(x, Wq, Wk, Wv, Wo, gamma, beta):
    x = np.asarray(x, dtype=np.float32)
    B, C = x.shape[0], x.shape[1]
    xf = np.ascontiguousarray(x.reshape(B, C, -1))
    xf16 = xf.astype(np.float16)
    Wq = np.asarray(Wq, dtype=np.float32)
    Wk = np.asarray(Wk, dtype=np.float32)
    WvT = np.asarray(Wv, dtype=np.float32).T
    WoT = np.asarray(Wo, dtype=np.float32).T
    g = np.asarray(gamma, dtype=np.float32).reshape(P, 1)
    b = np.asarray(beta, dtype=np.float32).reshape(P, 1)
    ident = np.eye(P, dtype=np.float32).astype(NPBF16)
    gs = np.zeros((P, NGROUPS), dtype=np.float32)
    gs[np.arange(P), np.arange(P) // (P // NGROUPS)] = 1.0
    gsT = np.ascontiguousarray(gs.T)
    pbm = np.ascontiguousarray(
        np.concatenate([Wq, Wk, WvT, WoT], axis=1)).astype(np.float16)
    assert pbm.shape == (P, NPB)
    in_maps = []
    for core in range(NCORES):
        bi, s = core // 4, core % 4
        bsel = np.zeros((P, 2), dtype=np.float32)
        bsel[:, bi] = 1.0
        pfm = np.ascontiguousarray(
            np.concatenate([gs * (1.0 / CNT), g, b, bsel],
                           axis=1)).astype(np.float32)
        assert pfm.shape == (P, NPF)
        in_maps.append({
            "xb": xf16[bi],
            "xs16": np.ascontiguousarray(xf16[bi][:, s * NS:(s + 1) * NS]),
            "xs": np.ascontiguousarray(xf[bi][:, s * NS:(s + 1) * NS]),
            "pb": pbm, "idb": ident, "pf": pfm, "gselT": gsT,
        })
    return in_maps


def assemble(results, spatial=(16, 16, 16)):
    y = np.empty((2, P, N), dtype=np.float32)
    for core in range(NCORES):
        bi, s = core // 4, core % 4
        y[bi][:, s * NS:(s + 1) * NS] = results[core]["out"]
    return y.reshape(2, P, *spatial)


def kernel(x, Wq, Wk, Wv, Wo, gamma, beta):
    nc = _get_nc()
    in_maps = make_in_maps(x, Wq, Wk, Wv, Wo, gamma, beta)
    res = run_bass_kernel_spmd(nc, in_maps, list(range(NCORES)))
    return assemble(res.results, spatial=tuple(np.asarray(x).shape[2:]))


# revision 18
# speedup vs baseline: 1.0677x; 1.0677x over previous
"""Trainium2 Bass kernel for nn_Attention_73718818669284.

Reference computation (per batch b of 2, C=128 channels, N=4096 spatial):
    q = Wq x, k = Wk x, v = Wv x           (1x1 conv == channel matmul)
    w = softmax(q^T k, axis=-1)            ([N, N] attention)
    h = Wo (v w^T)
    y = x + h
    out = SiLU(GroupNorm8(y) * gamma + beta)

Sharding: 8 cores = 2 batches x 4 column-slices of N (1024 each).
Each core computes its slice of the attention output; GroupNorm statistics
are combined across the 4 cores of a batch with a tiny AllReduce (a
warm-up collective at kernel entry wakes the CC cores and absorbs the
cross-core start stagger in parallel with the prologue DMAs).

Per-core algorithm (transposed-score layout -> no PE transposes of P):
    M   = Wq^T Wk                     (one 128x128 matmul, fp16)
    R   = M^T X_s                     ([128, 1024] fp16, folds q-projection)
    S^T chunk j = X[:,128j:]^T R      ([128m, 1024n]; scores, fp16 inputs)
    P^T = exp(S^T + shift)            (bf16; shift cancels in softmax)
    rowsum = sum_m P^T[m, n]          (DVE bf16 dual accumulators)
    h_un = V P = sum_j VT_j^T PT_j    (V^T via PE transpose mode, bf16)
    h = h_un * (1/rowsum)             (1/r = exp(-ln r) on ACT, set 6)
    y = Wo h + x_s ; stats exchange; GroupNorm; SiLU.

Matmul dtypes: the score path (X, M, R, Wo/h) runs in fp16 (10-bit
mantissa keeps score errors ~1e-3; bf16 scores measure 3e-2 rel err),
the P-side (exp output, V, rowsums) in bf16 (fp16 would overflow at
e^41). Both stream 1 cycle/row with fast weight load; the fp32 moving
path measures ~2x slower. The ACT table set 6 (exp+ln+square) is pinned
at kernel entry; the only switch (silu) hides under the stats exchange.
GroupNorm rstd uses the DVE rsqrt bit-trick + 2 Newton steps.
"""

import numpy as np

import concourse.bass as bass
import concourse.tile as tile
from concourse import bacc, mybir
from concourse.bass_utils import run_bass_kernel_spmd

F32 = mybir.dt.float32
F16 = mybir.dt.float16
BF16 = mybir.dt.bfloat16
NPBF16 = mybir.dt.np(mybir.dt.bfloat16)
AF = mybir.ActivationFunctionType
ALU = mybir.AluOpType
AX = mybir.AxisListType

P = 128          # channels / partitions
N = 4096         # spatial size (16*16*16)
NS = 1024        # per-core slice of N
NB = N // P      # 32 m-chunks
NCORES = 8
NGROUPS = 8
EPS = 1e-5
CNT = (P // NGROUPS) * N   # elements per group per batch = 16 * 4096
NPB = 4 * P                # fp16 params width (wq|wk|wvT|woT)
NPF = NGROUPS + 4          # fp32 params width (gsel/CNT | gamma | beta | bsel0 | bsel1)
ACT_SET_MAIN = 6           # natural_log_exp_and_others: exp + ln + square


def _load_act_set(nc, set_id):
    return nc.scalar.add_instruction(
        mybir.InstLoadActFuncSet(
            name=nc.get_next_instruction_name(),
            ins=[], outs=[],
            act_func_set_id=set_id,
        )
    )


def _build_nc():
    nc = bacc.Bacc("TRN2", target_bir_lowering=False, debug=False,
                   num_devices=NCORES)

    xb = nc.declare_dram_parameter("xb", [P, N], F16, isOutput=False)
    xs16 = nc.declare_dram_parameter("xs16", [P, NS], F16, isOutput=False)
    xs = nc.declare_dram_parameter("xs", [P, NS], F32, isOutput=False)
    pb = nc.declare_dram_parameter("pb", [P, NPB], F16, isOutput=False)
    idb = nc.declare_dram_parameter("idb", [P, P], BF16, isOutput=False)
    pf = nc.declare_dram_parameter("pf", [P, NPF], F32, isOutput=False)
    gselT = nc.declare_dram_parameter("gselT", [NGROUPS, P], F32,
                                      isOutput=False)
    out = nc.declare_dram_parameter("out", [P, NS], F32, isOutput=True)

    with tile.TileContext(nc) as tc:
        _emit(nc, tc, xb, xs16, xs, pb, idb, pf, gselT, out)
    nc.compile()
    return nc


def _emit(nc, tc, xb, xs16, xs, pb, idb, pf, gselT, out):
    with (
        tc.tile_pool(name="pp", bufs=1) as pp,
        tc.tile_pool(name="ptp", bufs=4) as ptp,
        tc.tile_pool(name="dp", bufs=1, space="DRAM") as dp,
    ):
        # Pin the exp+ln+square table set before any ACT op; every
        # loop/epilogue activation is then satisfied and the only
        # remaining switch (silu) hides under the stats exchange.
        _load_act_set(nc, ACT_SET_MAIN)

        # ---------------- loads (two HWDGE rings in parallel) -----------
        pb_sb = pp.tile([P, NPB], F16)
        nc.scalar.dma_start(out=pb_sb[:], in_=pb[:])
        xsr = pp.tile([P, NS], F16)
        nc.scalar.dma_start(out=xsr[:], in_=xs16[:])
        id_sb = pp.tile([P, P], BF16)
        nc.scalar.dma_start(out=id_sb[:], in_=idb[:])
        pf_sb = pp.tile([P, NPF], F32)
        nc.scalar.dma_start(out=pf_sb[:], in_=pf[:])
        gselT_sb = pp.tile([NGROUPS, P], F32)
        nc.scalar.dma_start(out=gselT_sb[:], in_=gselT[:])
        # warm-up collective: aligns core start (prevents a fast core's
        # remote stats write racing a slow core's semaphore clear) and
        # wakes the CC cores in parallel with the prologue DMAs
        warm = pp.tile([1, 2], F32)
        nc.vector.memset(warm[:], 0.0)
        dumc_in = dp.tile([1, 2], F32)
        dumc_out = dp.tile([1, 2], F32)
        nc.sync.dma_start(out=dumc_in[:], in_=warm[:])
        nc.gpsimd.collective_compute(
            "AllReduce", ALU.add,
            replica_groups=[[0, 1, 2, 3, 4, 5, 6, 7]],
            ins=[dumc_in.opt()], outs=[dumc_out.opt()],
        )
        xs_sb = pp.tile([P, NS], F32)
        nc.scalar.dma_start(out=xs_sb[:], in_=xs[:])
        xr = pp.tile([P, N], F16)
        for i in range(4):
            nc.sync.dma_start(out=xr[:, i * NS:(i + 1) * NS],
                              in_=xb[:, i * NS:(i + 1) * NS])
        wq_b = pb_sb[:, 0:128]
        wk_b = pb_sb[:, 128:256]
        wvT_b = pb_sb[:, 256:384]
        woT_b = pb_sb[:, 384:512]
        gsel_c = pf_sb[:, 0:NGROUPS]        # scaled by 1/CNT host-side
        gamma_sb = pf_sb[:, NGROUPS:NGROUPS + 1]
        beta_sb = pf_sb[:, NGROUPS + 1:NGROUPS + 2]
        bsel0 = pf_sb[:, NGROUPS + 2:NGROUPS + 3]   # 1.0 iff batch-0 core
        bsel1 = pf_sb[:, NGROUPS + 3:NGROUPS + 4]   # 1.0 iff batch-1 core

        gselT_c = pp.tile([NGROUPS, P], F32)
        nc.vector.tensor_copy(gselT_c[:], gselT_sb[:])
        onesM = pp.tile([P, P], BF16)
        nc.vector.memset(onesM[:], 1.0)
        # Global exp shift: cancels exactly in softmax. Centers the
        # log-rowsum range [21.6, 103.5] inside exp/ln's clean window.
        shift = pp.tile([P, 1], F32)
        nc.vector.memset(shift[:], -62.5)

        stat_sb = pp.tile([P, 2], F32)

        # ------------- projections + attention loop (interleaved) -------
        r_r = pp.tile([P, NS], F16)
        v_sb = pp.tile([P, N], BF16)
        vt_sb = pp.tile([P, NB, P], BF16)
        h_sb = pp.tile([P, NS], F16)
        rsA = pp.tile([P, NS], BF16)
        rsB = pp.tile([P, NS], BF16)
        with (
            tc.tile_pool(name="stp", bufs=2, space="PSUM") as stp,
            tc.tile_pool(name="acc", bufs=1, space="PSUM") as acc,
        ):
            h_ps = acc.tile([P, NS], F32, tag="h")

            # M = Wq^T Wk  -> R = M^T Xs
            at_ps = stp.tile([P, P], F32, tag="st", name="at_ps")
            nc.tensor.matmul(at_ps[:], wq_b, wk_b, start=True, stop=True)
            at_b = pp.tile([P, P], F16)
            nc.vector.tensor_copy(at_b[:], at_ps[:])
            r_ps = stp.tile([P, NS], F32, tag="st", name="r_ps")
            nc.tensor.matmul(r_ps[:, 0:512], at_b[:], xsr[:, 0:512],
                             start=True, stop=True)
            nc.tensor.matmul(r_ps[:, 512:NS], at_b[:], xsr[:, 512:NS],
                             start=True, stop=True)
            nc.vector.tensor_copy(r_r[:, 0:512], r_ps[:, 0:512])
            nc.vector.tensor_copy(r_r[:, 512:NS], r_ps[:, 512:NS])

            def emit_vgroup(g):
                # V chunk g = Wv X[:, 512g:512g+512], then 4 PE transposes
                v_ps = stp.tile([P, 512], F32, tag="v", bufs=1,
                                name=f"v_ps{g}")
                nc.tensor.matmul(v_ps[:], wvT_b,
                                 xr[:, 512 * g:512 * (g + 1)],
                                 start=True, stop=True)
                nc.vector.tensor_copy(v_sb[:, 512 * g:512 * (g + 1)], v_ps[:])
                vt_ps = stp.tile([P, 4, P], BF16, tag="vt", bufs=1,
                                 name=f"vt_ps{g}")
                for t in range(4):
                    jj = 4 * g + t
                    nc.tensor.transpose(vt_ps[:, t, :],
                                        v_sb[:, jj * P:(jj + 1) * P], id_sb[:])
                nc.vector.tensor_copy(vt_sb[:, 4 * g:4 * g + 4, :], vt_ps[:])

            def consume(jj, ptj):
                first = jj == 0
                last = jj == NB - 1
                nc.tensor.matmul(h_ps[:, 0:512], vt_sb[:, jj, :], ptj[:, 0:512],
                                 start=first, stop=last)
                nc.tensor.matmul(h_ps[:, 512:NS], vt_sb[:, jj, :], ptj[:, 512:NS],
                                 start=first, stop=last)

            def rs_add(jj, ptj):
                # dual bf16 accumulators: 2x DVE mode, halved error depth
                dst = rsA if jj % 2 == 0 else rsB
                if jj < 2:
                    nc.vector.tensor_copy(dst[:], ptj[:])
                else:
                    nc.vector.tensor_add(dst[:], dst[:], ptj[:])

            # scores start immediately (need only xr chunk 0 + R); V/V^T
            # groups are woven into every 4th early iteration; PV matmuls lag
            # two iterations, the DVE row-sum adds lag one.
            vg_at = {2 + 4 * g: g for g in range(8)}   # j -> group
            pts = []
            for j in range(NB):
                if j in vg_at:
                    emit_vgroup(vg_at[j])
                st_ps = stp.tile([P, NS], F32, tag="st", name=f"st_ps{j}")
                lhs = xr[:, j * P:(j + 1) * P]
                nc.tensor.matmul(st_ps[:, 0:512], lhs, r_r[:, 0:512],
                                 start=True, stop=True)
                nc.tensor.matmul(st_ps[:, 512:NS], lhs, r_r[:, 512:NS],
                                 start=True, stop=True)
                pt = ptp.tile([P, NS], BF16, tag="pt", name=f"pt{j}")
                nc.scalar.activation(pt[:], st_ps[:], AF.Exp, bias=shift[:])
                pts.append(pt)
                if j >= 2:
                    consume(j - 2, pts[j - 2])
                if j >= 1:
                    rs_add(j - 1, pts[j - 1])
            for jj in (NB - 2, NB - 1):
                consume(jj, pts[jj])
            rs_add(NB - 1, pts[NB - 1])

            # broadcast-fold both accumulators with an all-ones stationary,
            # summing them in PSUM: rb[p, n] = rowsum[n] on every partition.
            rb_ps = stp.tile([P, NS], F32, tag="st", name="rb_ps")
            nc.tensor.matmul(rb_ps[:, 0:512], onesM[:], rsA[:, 0:512],
                             start=True, stop=False)
            nc.tensor.matmul(rb_ps[:, 0:512], onesM[:], rsB[:, 0:512],
                             start=False, stop=True)
            nc.tensor.matmul(rb_ps[:, 512:NS], onesM[:], rsA[:, 512:NS],
                             start=True, stop=False)
            nc.tensor.matmul(rb_ps[:, 512:NS], onesM[:], rsB[:, 512:NS],
                             start=False, stop=True)

            # 1/rowsum = exp(-ln(rowsum)): both in the pinned table set;
            # covers the whole fp32 range unlike the ACT reciprocal.
            lnr = pp.tile([P, NS], F32)
            nc.scalar.activation(lnr[:], rb_ps[:], AF.Ln)
            rbinv = pp.tile([P, NS], F32)
            nc.scalar.activation(rbinv[:], lnr[:], AF.Exp, scale=-1.0)

            # h = h_un / rowsum (fp16 for the Wo matmul), by halves so the
            # Wo matmul overlaps the second multiply
            nc.vector.tensor_mul(h_sb[:, 0:512], h_ps[:, 0:512],
                                 rbinv[:, 0:512])
            nc.vector.tensor_mul(h_sb[:, 512:NS], h_ps[:, 512:NS],
                                 rbinv[:, 512:NS])

        # ------------- output projection + residual + GroupNorm + SiLU ----
        with tc.tile_pool(name="ep", bufs=1, space="PSUM") as ep:
            a_ps = ep.tile([P, NS], F32, tag="a")
            nc.tensor.matmul(a_ps[:, 0:512], woT_b, h_sb[:, 0:512],
                             start=True, stop=True)
            nc.tensor.matmul(a_ps[:, 512:NS], woT_b, h_sb[:, 512:NS],
                             start=True, stop=True)
            y_sb = pp.tile([P, NS], F32)
            nc.vector.tensor_add(y_sb[:, 0:512], a_ps[:, 0:512],
                                 xs_sb[:, 0:512])
            nc.vector.tensor_add(y_sb[:, 512:NS], a_ps[:, 512:NS],
                                 xs_sb[:, 512:NS])

            # per-channel partial stats over the local 1024 columns; halves
            # so the first half's reductions overlap the second half's add
            hsum = pp.tile([P, 2], F32)
            nc.vector.reduce_sum(hsum[:, 0:1], y_sb[:, 0:512], axis=AX.X)
            sq_sb = pp.tile([P, NS], F32)
            nc.scalar.activation(sq_sb[:, 0:512], y_sb[:, 0:512], AF.Square,
                                 accum_out=hsum[:, 1:2])
            hsum2 = pp.tile([P, 2], F32)
            nc.vector.reduce_sum(hsum2[:, 0:1], y_sb[:, 512:NS], axis=AX.X)
            nc.scalar.activation(sq_sb[:, 512:NS], y_sb[:, 512:NS], AF.Square,
                                 accum_out=hsum2[:, 1:2])
            nc.vector.tensor_add(stat_sb[:], hsum[:], hsum2[:])

            # ONE 8-rank AllReduce on a batch-masked [128, 4] payload:
            # cols 0-1 carry this core's stats if it is a batch-0 core,
            # cols 2-3 if batch-1. Two concurrent 4-rank group collectives
            # serialize on the CC machinery (~15us extra for the second
            # group); a single 8-rank op avoids that. Each core selects its
            # batch's half post-reduce. Silu table set preloads in flight.
            ms_sb = pp.tile([P, 4], F32)
            nc.vector.tensor_scalar(out=ms_sb[:, 0:2], in0=stat_sb[:],
                                    scalar1=bsel0, scalar2=None,
                                    op0=ALU.mult)
            nc.vector.tensor_scalar(out=ms_sb[:, 2:4], in0=stat_sb[:],
                                    scalar1=bsel1, scalar2=None,
                                    op0=ALU.mult)
            d_st1 = dp.tile([P, 4], F32)
            d_st2 = dp.tile([P, 4], F32)
            nc.sync.dma_start(out=d_st1[:], in_=ms_sb[:])
            dumo = pp.tile([1, 1], F32)
            nc.scalar.activation(dumo[:], stat_sb[0:1, 0:1], AF.Silu)
            nc.gpsimd.collective_compute(
                "AllReduce", ALU.add,
                replica_groups=[[0, 1, 2, 3, 4, 5, 6, 7]],
                ins=[d_st1.opt()], outs=[d_st2.opt()],
            )
            ast_sb = pp.tile([P, 4], F32)
            nc.sync.dma_start(out=ast_sb[:], in_=d_st2[:])
            sel0 = pp.tile([P, 2], F32)
            nc.vector.tensor_scalar(out=sel0[:], in0=ast_sb[:, 0:2],
                                    scalar1=bsel0, scalar2=None,
                                    op0=ALU.mult)
            sel1 = pp.tile([P, 2], F32)
            nc.vector.tensor_scalar(out=sel1[:], in0=ast_sb[:, 2:4],
                                    scalar1=bsel1, scalar2=None,
                                    op0=ALU.mult)
            ast_c = pp.tile([P, 2], F32)
            nc.vector.tensor_add(ast_c[:], sel0[:], sel1[:])

            # fold channels -> groups; gsel carries 1/CNT so this yields
            # [mean, E[y^2]] per group directly
            gs_ps = ep.tile([NGROUPS, 2], F32, tag="gs")
            nc.tensor.matmul(gs_ps[:], gsel_c, ast_c[:], start=True, stop=True)
            mg = pp.tile([NGROUPS, 2], F32)
            nc.vector.tensor_copy(mg[:], gs_ps[:])
            msq = pp.tile([NGROUPS, 1], F32)
            nc.vector.tensor_mul(msq[:], mg[:, 0:1], mg[:, 0:1])
            var8 = pp.tile([NGROUPS, 1], F32)
            nc.vector.tensor_sub(var8[:], mg[:, 1:2], msq[:])
            # rstd = 1/sqrt(var + eps) via bit-trick + 2 Newton steps on the
            # DVE ([8,1] tiles) — keeps the pinned ACT table set intact.
            ve8 = pp.tile([NGROUPS, 1], F32)
            nc.vector.tensor_scalar_add(ve8[:], in0=var8[:], scalar1=EPS)
            I32 = mybir.dt.int32
            magic = pp.tile([NGROUPS, 1], I32)
            nc.vector.memset(magic[:], 0x5F3759DF)
            ish = pp.tile([NGROUPS, 1], I32)
            nc.vector.tensor_scalar(out=ish[:], in0=ve8.bitcast(I32),
                                    scalar1=1, scalar2=None,
                                    op0=ALU.arith_shift_right)
            y0i = pp.tile([NGROUPS, 1], I32)
            nc.vector.tensor_sub(y0i[:], magic[:], ish[:])
            ycur = y0i.bitcast(F32)
            for it in range(2):
                yy = pp.tile([NGROUPS, 1], F32, name=f"yy{it}")
                nc.vector.tensor_mul(yy[:], ycur[:], ycur[:])
                vy2 = pp.tile([NGROUPS, 1], F32, name=f"vy2{it}")
                nc.vector.tensor_mul(vy2[:], ve8[:], yy[:])
                hh = pp.tile([NGROUPS, 1], F32, name=f"hh{it}")
                nc.vector.tensor_scalar(out=hh[:], in0=vy2[:], scalar1=-0.5,
                                        scalar2=1.5, op0=ALU.mult, op1=ALU.add)
                ynew = pp.tile([NGROUPS, 1], F32, name=f"ynew{it}")
                nc.vector.tensor_mul(ynew[:], ycur[:], hh[:])
                ycur = ynew
            gval = pp.tile([NGROUPS, 2], F32)
            nc.vector.tensor_copy(gval[:, 0:1], mg[:, 0:1])
            nc.vector.tensor_copy(gval[:, 1:2], ycur[:])

            # broadcast group stats back to channels: [128, 2] = G @ gval
            pc_ps = ep.tile([P, 2], F32, tag="pc")
            nc.tensor.matmul(pc_ps[:], gselT_c[:], gval[:], start=True, stop=True)
            pc_sb = pp.tile([P, 2], F32)
            nc.vector.tensor_copy(pc_sb[:], pc_ps[:])

            # fuse (y - mean)*rstd*gamma + beta into one pass:
            # A = rstd*gamma, B = beta - mean*A, z = y*A + B
            A_sb = pp.tile([P, 1], F32)
            nc.vector.tensor_mul(A_sb[:], pc_sb[:, 1:2], gamma_sb)
            t_sb = pp.tile([P, 1], F32)
            nc.vector.tensor_mul(t_sb[:], pc_sb[:, 0:1], A_sb[:])
            B_sb = pp.tile([P, 1], F32)
            nc.vector.tensor_sub(B_sb[:], beta_sb, t_sb[:])
            z_sb = pp.tile([P, NS], F32)
            nc.vector.tensor_scalar(out=z_sb[:], in0=y_sb[:],
                                    scalar1=A_sb[:], scalar2=B_sb[:],
                                    op0=ALU.mult, op1=ALU.add)
            o_sb = pp.tile([P, NS], F32)
            nc.scalar.activation(o_sb[:], z_sb[:], AF.Silu)
            nc.sync.dma_start(out=out[:], in_=o_sb[:])


_NC_CACHE = None


def _get_nc():
    global _NC_CACHE
    if _NC_CACHE is None:
        _NC_CACHE = _build_nc()
    return _NC_CACHE


def make_in_maps(x, Wq, Wk, Wv, Wo, gamma, beta):
    x = np.asarray(x, dtype=np.float32)
    B, C = x.shape[0], x.shape[1]
    xf = np.ascontiguousarray(x.reshape(B, C, -1))
    xf16 = xf.astype(np.float16)
    Wq = np.asarray(Wq, dtype=np.float32)
    Wk = np.asarray(Wk, dtype=np.float32)
    WvT = np.asarray(Wv, dtype=np.float32).T
    WoT = np.asarray(Wo, dtype=np.float32).T
    g = np.asarray(gamma, dtype=np.float32).reshape(P, 1)
    b = np.asarray(beta, dtype=np.float32).reshape(P, 1)
    ident = np.eye(P, dtype=np.float32).astype(NPBF16)
    gs = np.zeros((P, NGROUPS), dtype=np.float32)
    gs[np.arange(P), np.arange(P) // (P // NGROUPS)] = 1.0
    gsT = np.ascontiguousarray(gs.T)
    pbm = np.ascontiguousarray(
        np.concatenate([Wq, Wk, WvT, WoT], axis=1)).astype(np.float16)
    assert pbm.shape == (P, NPB)
    in_maps = []
    for core in range(NCORES):
        bi, s = core // 4, core % 4
        bsel = np.zeros((P, 2), dtype=np.float32)
        bsel[:, bi] = 1.0
        pfm = np.ascontiguousarray(
            np.concatenate([gs * (1.0 / CNT), g, b, bsel],
                           axis=1)).astype(np.float32)
        assert pfm.shape == (P, NPF)
        in_maps.append({
            "xb": xf16[bi],
            "xs16": np.ascontiguousarray(xf16[bi][:, s * NS:(s + 1) * NS]),
            "xs": np.ascontiguousarray(xf[bi][:, s * NS:(s + 1) * NS]),
            "pb": pbm, "idb": ident, "pf": pfm, "gselT": gsT,
        })
    return in_maps


def assemble(results, spatial=(16, 16, 16)):
    y = np.empty((2, P, N), dtype=np.float32)
    for core in range(NCORES):
        bi, s = core // 4, core % 4
        y[bi][:, s * NS:(s + 1) * NS] = results[core]["out"]
    return y.reshape(2, P, *spatial)


def kernel(x, Wq, Wk, Wv, Wo, gamma, beta):
    nc = _get_nc()
    in_maps = make_in_maps(x, Wq, Wk, Wv, Wo, gamma, beta)
    res = run_bass_kernel_spmd(nc, in_maps, list(range(NCORES)))
    return assemble(res.results, spatial=tuple(np.asarray(x).shape[2:]))


# revision 19
# speedup vs baseline: 1.1982x; 1.1222x over previous
"""Trainium2 Bass kernel for nn_Attention_73718818669284.

Reference computation (per batch b of 2, C=128 channels, N=4096 spatial):
    q = Wq x, k = Wk x, v = Wv x           (1x1 conv == channel matmul)
    w = softmax(q^T k, axis=-1)            ([N, N] attention)
    h = Wo (v w^T)
    y = x + h
    out = SiLU(GroupNorm8(y) * gamma + beta)

Sharding: 8 cores = 2 batches x 4 column-slices of N (1024 each).
Each core computes its slice of the attention output; GroupNorm statistics
are combined with ONE 8-rank AllReduce on a batch-masked [128, 4] payload
(each core contributes its stats in its batch's column pair and selects
its half post-reduce). Two concurrent 4-rank group collectives serialize
on the CC machinery (~15us extra for the second group); the single 8-rank
op avoids that. A warm-up collective at kernel entry wakes the CC cores
and absorbs cross-core start stagger in parallel with the prologue DMAs.

Per-core algorithm (transposed-score layout -> no PE transposes of P):
    M   = Wq^T Wk                     (one 128x128 matmul, fp16)
    R   = M^T X_s                     ([128, 1024] fp16, folds q-projection)
    S^T chunk j = X[:,128j:]^T R      ([128m, 1024n]; scores, fp16 inputs)
    P^T = exp(S^T + shift)            (bf16; shift cancels in softmax)
    rowsum = sum_m P^T[m, n]          (DVE bf16 dual accumulators)
    h_un = V P = sum_j VT_j^T PT_j    (V^T via PE transpose mode, bf16)
    h = h_un * (1/rowsum)             (1/r = exp(-ln r) on ACT, set 6)
    y = Wo h + x_s ; stats exchange; GroupNorm; SiLU.

Matmul dtypes: the score path (X, M, R, Wo/h) runs in fp16 (10-bit
mantissa keeps score errors ~1e-3; bf16 scores measure 3e-2 rel err),
the P-side (exp output, V, rowsums) in bf16 (fp16 would overflow at
e^41). Both stream 1 cycle/row with fast weight load; the fp32 moving
path measures ~2x slower. The ACT table set 6 (exp+ln+square) is pinned
at kernel entry; the only switch (silu) hides under the stats exchange.
GroupNorm rstd uses the DVE rsqrt bit-trick + 2 Newton steps.
"""

import numpy as np

import concourse.bass as bass
import concourse.tile as tile
from concourse import bacc, mybir
from concourse.bass_utils import run_bass_kernel_spmd

F32 = mybir.dt.float32
F16 = mybir.dt.float16
BF16 = mybir.dt.bfloat16
NPBF16 = mybir.dt.np(mybir.dt.bfloat16)
AF = mybir.ActivationFunctionType
ALU = mybir.AluOpType
AX = mybir.AxisListType

P = 128          # channels / partitions
N = 4096         # spatial size (16*16*16)
NS = 1024        # per-core slice of N
NB = N // P      # 32 m-chunks
NCORES = 8
NGROUPS = 8
EPS = 1e-5
CNT = (P // NGROUPS) * N   # elements per group per batch = 16 * 4096
NPB = 4 * P                # fp16 params width (wq|wk|wvT|woT)
NPF = NGROUPS + 4          # fp32 params width (gsel/CNT | gamma | beta | bsel0 | bsel1)
ACT_SET_MAIN = 6           # natural_log_exp_and_others: exp + ln + square


def _load_act_set(nc, set_id):
    return nc.scalar.add_instruction(
        mybir.InstLoadActFuncSet(
            name=nc.get_next_instruction_name(),
            ins=[], outs=[],
            act_func_set_id=set_id,
        )
    )


def _build_nc():
    nc = bacc.Bacc("TRN2", target_bir_lowering=False, debug=False,
                   num_devices=NCORES)

    xb = nc.declare_dram_parameter("xb", [P, N], F16, isOutput=False)
    xs16 = nc.declare_dram_parameter("xs16", [P, NS], F16, isOutput=False)
    xs = nc.declare_dram_parameter("xs", [P, NS], F32, isOutput=False)
    pb = nc.declare_dram_parameter("pb", [P, NPB], F16, isOutput=False)
    idb = nc.declare_dram_parameter("idb", [P, P], BF16, isOutput=False)
    pf = nc.declare_dram_parameter("pf", [P, NPF], F32, isOutput=False)
    gselT = nc.declare_dram_parameter("gselT", [NGROUPS, P], F32,
                                      isOutput=False)
    out = nc.declare_dram_parameter("out", [P, NS], F32, isOutput=True)

    with tile.TileContext(nc) as tc:
        _emit(nc, tc, xb, xs16, xs, pb, idb, pf, gselT, out)
    nc.compile()
    return nc


def _emit(nc, tc, xb, xs16, xs, pb, idb, pf, gselT, out):
    with (
        tc.tile_pool(name="pp", bufs=1) as pp,
        tc.tile_pool(name="ptp", bufs=4) as ptp,
        tc.tile_pool(name="dp", bufs=1, space="DRAM") as dp,
    ):
        # Pin the exp+ln+square table set before any ACT op; every
        # loop/epilogue activation is then satisfied and the only
        # remaining switch (silu) hides under the stats exchange.
        _load_act_set(nc, ACT_SET_MAIN)

        # ---------------- loads (two HWDGE rings in parallel) -----------
        pb_sb = pp.tile([P, NPB], F16)
        nc.scalar.dma_start(out=pb_sb[:], in_=pb[:])
        xsr = pp.tile([P, NS], F16)
        nc.scalar.dma_start(out=xsr[:], in_=xs16[:])
        id_sb = pp.tile([P, P], BF16)
        nc.scalar.dma_start(out=id_sb[:], in_=idb[:])
        pf_sb = pp.tile([P, NPF], F32)
        nc.scalar.dma_start(out=pf_sb[:], in_=pf[:])
        gselT_sb = pp.tile([NGROUPS, P], F32)
        nc.scalar.dma_start(out=gselT_sb[:], in_=gselT[:])
        # warm-up collective: aligns core start (prevents a fast core's
        # remote stats write racing a slow core's semaphore clear) and
        # wakes the CC cores in parallel with the prologue DMAs
        warm = pp.tile([1, 2], F32)
        nc.vector.memset(warm[:], 0.0)
        dumc_in = dp.tile([1, 2], F32)
        dumc_out = dp.tile([1, 2], F32)
        nc.sync.dma_start(out=dumc_in[:], in_=warm[:])
        nc.gpsimd.collective_compute(
            "AllReduce", ALU.add,
            replica_groups=[[0, 1, 2, 3, 4, 5, 6, 7]],
            ins=[dumc_in.opt()], outs=[dumc_out.opt()],
        )
        xs_sb = pp.tile([P, NS], F32)
        nc.scalar.dma_start(out=xs_sb[:], in_=xs[:])
        xr = pp.tile([P, N], F16)
        for i in range(4):
            nc.sync.dma_start(out=xr[:, i * NS:(i + 1) * NS],
                              in_=xb[:, i * NS:(i + 1) * NS])
        wq_b = pb_sb[:, 0:128]
        wk_b = pb_sb[:, 128:256]
        wvT_b = pb_sb[:, 256:384]
        woT_b = pb_sb[:, 384:512]
        gsel_c = pf_sb[:, 0:NGROUPS]        # scaled by 1/CNT host-side
        gamma_sb = pf_sb[:, NGROUPS:NGROUPS + 1]
        beta_sb = pf_sb[:, NGROUPS + 1:NGROUPS + 2]
        bsel0 = pf_sb[:, NGROUPS + 2:NGROUPS + 3]   # 1.0 iff batch-0 core
        bsel1 = pf_sb[:, NGROUPS + 3:NGROUPS + 4]   # 1.0 iff batch-1 core

        gselT_c = pp.tile([NGROUPS, P], F32)
        nc.vector.tensor_copy(gselT_c[:], gselT_sb[:])
        onesM = pp.tile([P, P], BF16)
        nc.vector.memset(onesM[:], 1.0)
        # Global exp shift: cancels exactly in softmax. Centers the
        # log-rowsum range [21.6, 103.5] inside exp/ln's clean window.
        shift = pp.tile([P, 1], F32)
        nc.vector.memset(shift[:], -62.5)

        stat_sb = pp.tile([P, 2], F32)

        # ------------- projections + attention loop (interleaved) -------
        r_r = pp.tile([P, NS], F16)
        v_sb = pp.tile([P, N], BF16)
        vt_sb = pp.tile([P, NB, P], BF16)
        h_sb = pp.tile([P, NS], F16)
        rsA = pp.tile([P, NS], BF16)
        rsB = pp.tile([P, NS], BF16)
        with (
            tc.tile_pool(name="stp", bufs=2, space="PSUM") as stp,
            tc.tile_pool(name="acc", bufs=1, space="PSUM") as acc,
        ):
            h_ps = acc.tile([P, NS], F32, tag="h")

            # M = Wq^T Wk  -> R = M^T Xs
            at_ps = stp.tile([P, P], F32, tag="st", name="at_ps")
            nc.tensor.matmul(at_ps[:], wq_b, wk_b, start=True, stop=True)
            at_b = pp.tile([P, P], F16)
            nc.vector.tensor_copy(at_b[:], at_ps[:])
            r_ps = stp.tile([P, NS], F32, tag="st", name="r_ps")
            nc.tensor.matmul(r_ps[:, 0:512], at_b[:], xsr[:, 0:512],
                             start=True, stop=True)
            nc.tensor.matmul(r_ps[:, 512:NS], at_b[:], xsr[:, 512:NS],
                             start=True, stop=True)
            nc.vector.tensor_copy(r_r[:, 0:512], r_ps[:, 0:512])
            nc.vector.tensor_copy(r_r[:, 512:NS], r_ps[:, 512:NS])

            def emit_vgroup(g):
                # V chunk g = Wv X[:, 512g:512g+512], then 4 PE transposes
                v_ps = stp.tile([P, 512], F32, tag="v", bufs=1,
                                name=f"v_ps{g}")
                nc.tensor.matmul(v_ps[:], wvT_b,
                                 xr[:, 512 * g:512 * (g + 1)],
                                 start=True, stop=True)
                nc.vector.tensor_copy(v_sb[:, 512 * g:512 * (g + 1)], v_ps[:])
                vt_ps = stp.tile([P, 4, P], BF16, tag="vt", bufs=1,
                                 name=f"vt_ps{g}")
                for t in range(4):
                    jj = 4 * g + t
                    nc.tensor.transpose(vt_ps[:, t, :],
                                        v_sb[:, jj * P:(jj + 1) * P], id_sb[:])
                nc.vector.tensor_copy(vt_sb[:, 4 * g:4 * g + 4, :], vt_ps[:])

            def consume(jj, ptj):
                first = jj == 0
                last = jj == NB - 1
                nc.tensor.matmul(h_ps[:, 0:512], vt_sb[:, jj, :], ptj[:, 0:512],
                                 start=first, stop=last)
                nc.tensor.matmul(h_ps[:, 512:NS], vt_sb[:, jj, :], ptj[:, 512:NS],
                                 start=first, stop=last)

            def rs_add(jj, ptj):
                # dual bf16 accumulators: 2x DVE mode, halved error depth
                dst = rsA if jj % 2 == 0 else rsB
                if jj < 2:
                    nc.vector.tensor_copy(dst[:], ptj[:])
                else:
                    nc.vector.tensor_add(dst[:], dst[:], ptj[:])

            # scores start immediately (need only xr chunk 0 + R); V/V^T
            # groups are woven into every 4th early iteration; PV matmuls lag
            # two iterations, the DVE row-sum adds lag one.
            vg_at = {2 + 4 * g: g for g in range(8)}   # j -> group
            pts = []
            for j in range(NB):
                if j in vg_at:
                    emit_vgroup(vg_at[j])
                st_ps = stp.tile([P, NS], F32, tag="st", name=f"st_ps{j}")
                lhs = xr[:, j * P:(j + 1) * P]
                nc.tensor.matmul(st_ps[:, 0:512], lhs, r_r[:, 0:512],
                                 start=True, stop=True)
                nc.tensor.matmul(st_ps[:, 512:NS], lhs, r_r[:, 512:NS],
                                 start=True, stop=True)
                pt = ptp.tile([P, NS], BF16, tag="pt", name=f"pt{j}")
                nc.scalar.activation(pt[:], st_ps[:], AF.Exp, bias=shift[:])
                pts.append(pt)
                if j >= 2:
                    consume(j - 2, pts[j - 2])
                if j >= 1:
                    rs_add(j - 1, pts[j - 1])
            for jj in (NB - 2, NB - 1):
                consume(jj, pts[jj])
            rs_add(NB - 1, pts[NB - 1])

            # broadcast-fold both accumulators with an all-ones stationary,
            # summing them in PSUM: rb[p, n] = rowsum[n] on every partition.
            rb_ps = stp.tile([P, NS], F32, tag="st", name="rb_ps")
            nc.tensor.matmul(rb_ps[:, 0:512], onesM[:], rsA[:, 0:512],
                             start=True, stop=False)
            nc.tensor.matmul(rb_ps[:, 0:512], onesM[:], rsB[:, 0:512],
                             start=False, stop=True)
            nc.tensor.matmul(rb_ps[:, 512:NS], onesM[:], rsA[:, 512:NS],
                             start=True, stop=False)
            nc.tensor.matmul(rb_ps[:, 512:NS], onesM[:], rsB[:, 512:NS],
                             start=False, stop=True)

            # 1/rowsum = exp(-ln(rowsum)): both in the pinned table set;
            # covers the whole fp32 range unlike the ACT reciprocal.
            lnr = pp.tile([P, NS], F32)
            nc.scalar.activation(lnr[:], rb_ps[:], AF.Ln)
            rbinv = pp.tile([P, NS], F32)
            nc.scalar.activation(rbinv[:], lnr[:], AF.Exp, scale=-1.0)

            # h = h_un / rowsum (fp16 for the Wo matmul), by halves so the
            # Wo matmul overlaps the second multiply
            nc.vector.tensor_mul(h_sb[:, 0:512], h_ps[:, 0:512],
                                 rbinv[:, 0:512])
            nc.vector.tensor_mul(h_sb[:, 512:NS], h_ps[:, 512:NS],
                                 rbinv[:, 512:NS])

        # ------------- output projection + residual + GroupNorm + SiLU ----
        with tc.tile_pool(name="ep", bufs=1, space="PSUM") as ep:
            a_ps = ep.tile([P, NS], F32, tag="a")
            nc.tensor.matmul(a_ps[:, 0:512], woT_b, h_sb[:, 0:512],
                             start=True, stop=True)
            nc.tensor.matmul(a_ps[:, 512:NS], woT_b, h_sb[:, 512:NS],
                             start=True, stop=True)
            y_sb = pp.tile([P, NS], F32)
            nc.vector.tensor_add(y_sb[:, 0:512], a_ps[:, 0:512],
                                 xs_sb[:, 0:512])
            nc.vector.tensor_add(y_sb[:, 512:NS], a_ps[:, 512:NS],
                                 xs_sb[:, 512:NS])

            # per-channel partial stats over the local 1024 columns; halves
            # so the first half's reductions overlap the second half's add
            hsum = pp.tile([P, 2], F32)
            nc.vector.reduce_sum(hsum[:, 0:1], y_sb[:, 0:512], axis=AX.X)
            sq_sb = pp.tile([P, NS], F32)
            nc.scalar.activation(sq_sb[:, 0:512], y_sb[:, 0:512], AF.Square,
                                 accum_out=hsum[:, 1:2])
            hsum2 = pp.tile([P, 2], F32)
            nc.vector.reduce_sum(hsum2[:, 0:1], y_sb[:, 512:NS], axis=AX.X)
            nc.scalar.activation(sq_sb[:, 512:NS], y_sb[:, 512:NS], AF.Square,
                                 accum_out=hsum2[:, 1:2])
            nc.vector.tensor_add(stat_sb[:], hsum[:], hsum2[:])

            # ONE 8-rank AllReduce on a batch-masked [128, 4] payload:
            # cols 0-1 carry this core's stats if it is a batch-0 core,
            # cols 2-3 if batch-1. Two concurrent 4-rank group collectives
            # serialize on the CC machinery (~15us extra for the second
            # group); a single 8-rank op avoids that. Each core selects its
            # batch's half post-reduce. Silu table set preloads in flight.
            ms_sb = pp.tile([P, 4], F32)
            nc.vector.tensor_scalar(out=ms_sb[:, 0:2], in0=stat_sb[:],
                                    scalar1=bsel0, scalar2=None,
                                    op0=ALU.mult)
            nc.vector.tensor_scalar(out=ms_sb[:, 2:4], in0=stat_sb[:],
                                    scalar1=bsel1, scalar2=None,
                                    op0=ALU.mult)
            d_st1 = dp.tile([P, 4], F32)
            d_st2 = dp.tile([P, 4], F32)
            nc.sync.dma_start(out=d_st1[:], in_=ms_sb[:])
            dumo = pp.tile([1, 1], F32)
            nc.scalar.activation(dumo[:], stat_sb[0:1, 0:1], AF.Silu)
            nc.gpsimd.collective_compute(
                "AllReduce", ALU.add,
                replica_groups=[[0, 1, 2, 3, 4, 5, 6, 7]],
                ins=[d_st1.opt()], outs=[d_st2.opt()],
            )
            ast_sb = pp.tile([P, 4], F32)
            nc.sync.dma_start(out=ast_sb[:], in_=d_st2[:])
            sel0 = pp.tile([P, 2], F32)
            nc.vector.tensor_scalar(out=sel0[:], in0=ast_sb[:, 0:2],
                                    scalar1=bsel0, scalar2=None,
                                    op0=ALU.mult)
            sel1 = pp.tile([P, 2], F32)
            nc.vector.tensor_scalar(out=sel1[:], in0=ast_sb[:, 2:4],
                                    scalar1=bsel1, scalar2=None,
                                    op0=ALU.mult)
            ast_c = pp.tile([P, 2], F32)
            nc.vector.tensor_add(ast_c[:], sel0[:], sel1[:])

            # fold channels -> groups; gsel carries 1/CNT so this yields
            # [mean, E[y^2]] per group directly
            gs_ps = ep.tile([NGROUPS, 2], F32, tag="gs")
            nc.tensor.matmul(gs_ps[:], gsel_c, ast_c[:], start=True, stop=True)
            mg = pp.tile([NGROUPS, 2], F32)
            nc.vector.tensor_copy(mg[:], gs_ps[:])
            msq = pp.tile([NGROUPS, 1], F32)
            nc.vector.tensor_mul(msq[:], mg[:, 0:1], mg[:, 0:1])
            var8 = pp.tile([NGROUPS, 1], F32)
            nc.vector.tensor_sub(var8[:], mg[:, 1:2], msq[:])
            # rstd = 1/sqrt(var + eps) via bit-trick + 2 Newton steps on the
            # DVE ([8,1] tiles) — keeps the pinned ACT table set intact.
            ve8 = pp.tile([NGROUPS, 1], F32)
            nc.vector.tensor_scalar_add(ve8[:], in0=var8[:], scalar1=EPS)
            I32 = mybir.dt.int32
            magic = pp.tile([NGROUPS, 1], I32)
            nc.vector.memset(magic[:], 0x5F3759DF)
            ish = pp.tile([NGROUPS, 1], I32)
            nc.vector.tensor_scalar(out=ish[:], in0=ve8.bitcast(I32),
                                    scalar1=1, scalar2=None,
                                    op0=ALU.arith_shift_right)
            y0i = pp.tile([NGROUPS, 1], I32)
            nc.vector.tensor_sub(y0i[:], magic[:], ish[:])
            ycur = y0i.bitcast(F32)
            for it in range(2):
                yy = pp.tile([NGROUPS, 1], F32, name=f"yy{it}")
                nc.vector.tensor_mul(yy[:], ycur[:], ycur[:])
                vy2 = pp.tile([NGROUPS, 1], F32, name=f"vy2{it}")
                nc.vector.tensor_mul(vy2[:], ve8[:], yy[:])
                hh = pp.tile([NGROUPS, 1], F32, name=f"hh{it}")
                nc.vector.tensor_scalar(out=hh[:], in0=vy2[:], scalar1=-0.5,
                                        scalar2=1.5, op0=ALU.mult, op1=ALU.add)
                ynew = pp.tile([NGROUPS, 1], F32, name=f"ynew{it}")
                nc.vector.tensor_mul(ynew[:], ycur[:], hh[:])
                ycur = ynew
            gval = pp.tile([NGROUPS, 2], F32)
            nc.vector.tensor_copy(gval[:, 0:1], mg[:, 0:1])
            nc.vector.tensor_copy(gval[:, 1:2], ycur[:])

            # broadcast group stats back to channels: [128, 2] = G @ gval
            pc_ps = ep.tile([P, 2], F32, tag="pc")
            nc.tensor.matmul(pc_ps[:], gselT_c[:], gval[:], start=True, stop=True)
            pc_sb = pp.tile([P, 2], F32)
            nc.vector.tensor_copy(pc_sb[:], pc_ps[:])

            # fuse (y - mean)*rstd*gamma + beta into one pass:
            # A = rstd*gamma, B = beta - mean*A, z = y*A + B
            A_sb = pp.tile([P, 1], F32)
            nc.vector.tensor_mul(A_sb[:], pc_sb[:, 1:2], gamma_sb)
            t_sb = pp.tile([P, 1], F32)
            nc.vector.tensor_mul(t_sb[:], pc_sb[:, 0:1], A_sb[:])
            B_sb = pp.tile([P, 1], F32)
            nc.vector.tensor_sub(B_sb[:], beta_sb, t_sb[:])
            z_sb = pp.tile([P, NS], F32)
            nc.vector.tensor_scalar(out=z_sb[:], in0=y_sb[:],
                                    scalar1=A_sb[:], scalar2=B_sb[:],
                                    op0=ALU.mult, op1=ALU.add)
            o_sb = pp.tile([P, NS], F32)
            nc.scalar.activation(o_sb[:], z_sb[:], AF.Silu)
            nc.sync.dma_start(out=out[:], in_=o_sb[:])


_NC_CACHE = None


def _get_nc():
    global _NC_CACHE
    if _NC_CACHE is None:
        _NC_CACHE = _build_nc()
    return _NC_CACHE


def make_in_maps(x, Wq, Wk, Wv, Wo, gamma, beta):
    x = np.asarray(x, dtype=np.float32)
    B, C = x.shape[0], x.shape[1]
    xf = np.ascontiguousarray(x.reshape(B, C, -1))
    xf16 = xf.astype(np.float16)
    Wq = np.asarray(Wq, dtype=np.float32)
    Wk = np.asarray(Wk, dtype=np.float32)
    WvT = np.asarray(Wv, dtype=np.float32).T
    WoT = np.asarray(Wo, dtype=np.float32).T
    g = np.asarray(gamma, dtype=np.float32).reshape(P, 1)
    b = np.asarray(beta, dtype=np.float32).reshape(P, 1)
    ident = np.eye(P, dtype=np.float32).astype(NPBF16)
    gs = np.zeros((P, NGROUPS), dtype=np.float32)
    gs[np.arange(P), np.arange(P) // (P // NGROUPS)] = 1.0
    gsT = np.ascontiguousarray(gs.T)
    pbm = np.ascontiguousarray(
        np.concatenate([Wq, Wk, WvT, WoT], axis=1)).astype(np.float16)
    assert pbm.shape == (P, NPB)
    in_maps = []
    for core in range(NCORES):
        bi, s = core // 4, core % 4
        bsel = np.zeros((P, 2), dtype=np.float32)
        bsel[:, bi] = 1.0
        pfm = np.ascontiguousarray(
            np.concatenate([gs * (1.0 / CNT), g, b, bsel],
                           axis=1)).astype(np.float32)
        assert pfm.shape == (P, NPF)
        in_maps.append({
            "xb": xf16[bi],
            "xs16": np.ascontiguousarray(xf16[bi][:, s * NS:(s + 1) * NS]),
            "xs": np.ascontiguousarray(xf[bi][:, s * NS:(s + 1) * NS]),
            "pb": pbm, "idb": ident, "pf": pfm, "gselT": gsT,
        })
    return in_maps


def assemble(results, spatial=(16, 16, 16)):
    y = np.empty((2, P, N), dtype=np.float32)
    for core in range(NCORES):
        bi, s = core // 4, core % 4
        y[bi][:, s * NS:(s + 1) * NS] = results[core]["out"]
    return y.reshape(2, P, *spatial)


def kernel(x, Wq, Wk, Wv, Wo, gamma, beta):
    nc = _get_nc()
    in_maps = make_in_maps(x, Wq, Wk, Wv, Wo, gamma, beta)
    res = run_bass_kernel_spmd(nc, in_maps, list(range(NCORES)))
    return assemble(res.results, spatial=tuple(np.asarray(x).shape[2:]))


# revision 20
# speedup vs baseline: 1.2343x; 1.0301x over previous
"""Trainium2 Bass kernel for nn_Attention_73718818669284.

Reference computation (per batch b of 2, C=128 channels, N=4096 spatial):
    q = Wq x, k = Wk x, v = Wv x           (1x1 conv == channel matmul)
    w = softmax(q^T k, axis=-1)            ([N, N] attention)
    h = Wo (v w^T)
    y = x + h
    out = SiLU(GroupNorm8(y) * gamma + beta)

Sharding: 8 cores = 2 batches x 4 column-slices of N (1024 each).
Each core computes its slice of the attention output; GroupNorm statistics
are combined with ONE 8-rank AllReduce on a batch-masked [128, 4] payload
(each core contributes its stats in its batch's column pair and selects
its half post-reduce). Two concurrent 4-rank group collectives serialize
on the CC machinery (~15us extra for the second group); the single 8-rank
op avoids that. A warm-up collective at kernel entry wakes the CC cores
and absorbs cross-core start stagger in parallel with the prologue DMAs.

Per-core algorithm (transposed-score layout -> no PE transposes of P):
    M   = Wq^T Wk                     (one 128x128 matmul, fp16)
    R   = M^T X_s                     ([128, 1024] fp16, folds q-projection)
    S^T chunk j = X[:,128j:]^T R      ([128m, 1024n]; scores, fp16 inputs)
    P^T = exp(S^T + shift)            (bf16; shift cancels in softmax)
    rowsum = sum_m P^T[m, n]          (DVE bf16 dual accumulators)
    h_un = V P = sum_j VT_j^T PT_j    (V^T via PE transpose mode, bf16)
    h = h_un * (1/rowsum)             (1/r = exp(-ln r) on ACT, set 6)
    y = Wo h + x_s ; stats exchange; GroupNorm; SiLU.

Matmul dtypes: the score path (X, M, R, Wo/h) runs in fp16 (10-bit
mantissa keeps score errors ~1e-3; bf16 scores measure 3e-2 rel err),
the P-side (exp output, V, rowsums) in bf16 (fp16 would overflow at
e^41). Both stream 1 cycle/row with fast weight load; the fp32 moving
path measures ~2x slower. The ACT table set 6 (exp+ln+square) is pinned
at kernel entry; the only switch (silu) hides under the stats exchange.
GroupNorm rstd uses the DVE rsqrt bit-trick + 2 Newton steps.
"""

import numpy as np

import concourse.bass as bass
import concourse.tile as tile
from concourse import bacc, mybir
from concourse.bass_utils import run_bass_kernel_spmd

F32 = mybir.dt.float32
F16 = mybir.dt.float16
BF16 = mybir.dt.bfloat16
NPBF16 = mybir.dt.np(mybir.dt.bfloat16)
AF = mybir.ActivationFunctionType
ALU = mybir.AluOpType
AX = mybir.AxisListType

P = 128          # channels / partitions
N = 4096         # spatial size (16*16*16)
NS = 1024        # per-core slice of N
NB = N // P      # 32 m-chunks
NCORES = 8
NGROUPS = 8
EPS = 1e-5
CNT = (P // NGROUPS) * N   # elements per group per batch = 16 * 4096
NPB = 4 * P                # fp16 params width (wq|wk|wvT|woT)
NPF = NGROUPS + 4          # fp32 params width (gsel/CNT | gamma | beta | bsel0 | bsel1)
ACT_SET_MAIN = 6           # natural_log_exp_and_others: exp + ln + square


def _load_act_set(nc, set_id):
    return nc.scalar.add_instruction(
        mybir.InstLoadActFuncSet(
            name=nc.get_next_instruction_name(),
            ins=[], outs=[],
            act_func_set_id=set_id,
        )
    )


def _build_nc():
    nc = bacc.Bacc("TRN2", target_bir_lowering=False, debug=False,
                   num_devices=NCORES)

    xb = nc.declare_dram_parameter("xb", [P, N], F16, isOutput=False)
    xs16 = nc.declare_dram_parameter("xs16", [P, NS], F16, isOutput=False)
    xs = nc.declare_dram_parameter("xs", [P, NS], F32, isOutput=False)
    pb = nc.declare_dram_parameter("pb", [P, NPB], F16, isOutput=False)
    idb = nc.declare_dram_parameter("idb", [P, P], BF16, isOutput=False)
    pf = nc.declare_dram_parameter("pf", [P, NPF], F32, isOutput=False)
    gselT = nc.declare_dram_parameter("gselT", [NGROUPS, P], F32,
                                      isOutput=False)
    out = nc.declare_dram_parameter("out", [P, NS], F32, isOutput=True)

    with tile.TileContext(nc) as tc:
        _emit(nc, tc, xb, xs16, xs, pb, idb, pf, gselT, out)
    nc.compile()
    return nc


def _emit(nc, tc, xb, xs16, xs, pb, idb, pf, gselT, out):
    with (
        tc.tile_pool(name="pp", bufs=1) as pp,
        tc.tile_pool(name="ptp", bufs=4) as ptp,
        tc.tile_pool(name="dp", bufs=1, space="DRAM") as dp,
    ):
        # Pin the exp+ln+square table set before any ACT op; every
        # loop/epilogue activation is then satisfied and the only
        # remaining switch (silu) hides under the stats exchange.
        _load_act_set(nc, ACT_SET_MAIN)

        # ---------------- loads (two HWDGE rings in parallel) -----------
        pb_sb = pp.tile([P, NPB], F16)
        nc.scalar.dma_start(out=pb_sb[:], in_=pb[:])
        xsr = pp.tile([P, NS], F16)
        nc.scalar.dma_start(out=xsr[:], in_=xs16[:])
        id_sb = pp.tile([P, P], BF16)
        nc.scalar.dma_start(out=id_sb[:], in_=idb[:])
        pf_sb = pp.tile([P, NPF], F32)
        nc.scalar.dma_start(out=pf_sb[:], in_=pf[:])
        gselT_sb = pp.tile([NGROUPS, P], F32)
        nc.scalar.dma_start(out=gselT_sb[:], in_=gselT[:])
        # warm-up collective: aligns core start (prevents a fast core's
        # remote stats write racing a slow core's semaphore clear) and
        # wakes the CC cores in parallel with the prologue DMAs
        warm = pp.tile([1, 2], F32)
        nc.vector.memset(warm[:], 0.0)
        dumc_in = dp.tile([1, 2], F32)
        dumc_out = dp.tile([1, 2], F32)
        nc.sync.dma_start(out=dumc_in[:], in_=warm[:])
        nc.gpsimd.collective_compute(
            "AllReduce", ALU.add,
            replica_groups=[[0, 1, 2, 3, 4, 5, 6, 7]],
            ins=[dumc_in.opt()], outs=[dumc_out.opt()],
        )
        xs_sb = pp.tile([P, NS], F32)
        nc.scalar.dma_start(out=xs_sb[:], in_=xs[:])
        xr = pp.tile([P, N], F16)
        for i in range(4):
            nc.sync.dma_start(out=xr[:, i * NS:(i + 1) * NS],
                              in_=xb[:, i * NS:(i + 1) * NS])
        wq_b = pb_sb[:, 0:128]
        wk_b = pb_sb[:, 128:256]
        wvT_b = pb_sb[:, 256:384]
        woT_b = pb_sb[:, 384:512]
        gsel_c = pf_sb[:, 0:NGROUPS]        # scaled by 1/CNT host-side
        gamma_sb = pf_sb[:, NGROUPS:NGROUPS + 1]
        beta_sb = pf_sb[:, NGROUPS + 1:NGROUPS + 2]
        bsel0 = pf_sb[:, NGROUPS + 2:NGROUPS + 3]   # 1.0 iff batch-0 core
        bsel1 = pf_sb[:, NGROUPS + 3:NGROUPS + 4]   # 1.0 iff batch-1 core

        gselT_c = pp.tile([NGROUPS, P], F32)
        nc.vector.tensor_copy(gselT_c[:], gselT_sb[:])
        onesM = pp.tile([P, P], BF16)
        nc.vector.memset(onesM[:], 1.0)
        # Global exp shift: cancels exactly in softmax. Centers the
        # log-rowsum range [21.6, 103.5] inside exp/ln's clean window.
        shift = pp.tile([P, 1], F32)
        nc.vector.memset(shift[:], -62.5)

        stat_sb = pp.tile([P, 2], F32)

        # ------------- projections + attention loop (interleaved) -------
        r_r = pp.tile([P, NS], F16)
        v_sb = pp.tile([P, N], BF16)
        vt_sb = pp.tile([P, NB, P], BF16)
        h_sb = pp.tile([P, NS], F16)
        rsA = pp.tile([P, NS], BF16)
        rsB = pp.tile([P, NS], BF16)
        with (
            tc.tile_pool(name="stp", bufs=2, space="PSUM") as stp,
            tc.tile_pool(name="acc", bufs=1, space="PSUM") as acc,
        ):
            h_ps = acc.tile([P, NS], F32, tag="h")

            # M = Wq^T Wk  -> R = M^T Xs
            at_ps = stp.tile([P, P], F32, tag="st", name="at_ps")
            nc.tensor.matmul(at_ps[:], wq_b, wk_b, start=True, stop=True)
            at_b = pp.tile([P, P], F16)
            nc.vector.tensor_copy(at_b[:], at_ps[:])
            r_ps = stp.tile([P, NS], F32, tag="st", name="r_ps")
            nc.tensor.matmul(r_ps[:, 0:512], at_b[:], xsr[:, 0:512],
                             start=True, stop=True)
            nc.tensor.matmul(r_ps[:, 512:NS], at_b[:], xsr[:, 512:NS],
                             start=True, stop=True)
            nc.vector.tensor_copy(r_r[:, 0:512], r_ps[:, 0:512])
            nc.vector.tensor_copy(r_r[:, 512:NS], r_ps[:, 512:NS])

            def emit_vgroup(g):
                # V chunk g = Wv X[:, 512g:512g+512], then 4 PE transposes
                v_ps = stp.tile([P, 512], F32, tag="v", bufs=1,
                                name=f"v_ps{g}")
                nc.tensor.matmul(v_ps[:], wvT_b,
                                 xr[:, 512 * g:512 * (g + 1)],
                                 start=True, stop=True)
                nc.vector.tensor_copy(v_sb[:, 512 * g:512 * (g + 1)], v_ps[:])
                vt_ps = stp.tile([P, 4, P], BF16, tag="vt", bufs=1,
                                 name=f"vt_ps{g}")
                for t in range(4):
                    jj = 4 * g + t
                    nc.tensor.transpose(vt_ps[:, t, :],
                                        v_sb[:, jj * P:(jj + 1) * P], id_sb[:])
                nc.vector.tensor_copy(vt_sb[:, 4 * g:4 * g + 4, :], vt_ps[:])

            def consume(jj, ptj):
                first = jj == 0
                last = jj == NB - 1
                nc.tensor.matmul(h_ps[:, 0:512], vt_sb[:, jj, :], ptj[:, 0:512],
                                 start=first, stop=last)
                nc.tensor.matmul(h_ps[:, 512:NS], vt_sb[:, jj, :], ptj[:, 512:NS],
                                 start=first, stop=last)

            def rs_add(jj, ptj):
                # dual bf16 accumulators: 2x DVE mode, halved error depth
                dst = rsA if jj % 2 == 0 else rsB
                if jj < 2:
                    nc.vector.tensor_copy(dst[:], ptj[:])
                else:
                    nc.vector.tensor_add(dst[:], dst[:], ptj[:])

            # scores start immediately (need only xr chunk 0 + R); V/V^T
            # groups are woven into every 4th early iteration; PV matmuls lag
            # two iterations, the DVE row-sum adds lag one.
            vg_at = {2 + 4 * g: g for g in range(8)}   # j -> group
            pts = []
            for j in range(NB):
                if j in vg_at:
                    emit_vgroup(vg_at[j])
                st_ps = stp.tile([P, NS], F32, tag="st", name=f"st_ps{j}")
                lhs = xr[:, j * P:(j + 1) * P]
                nc.tensor.matmul(st_ps[:, 0:512], lhs, r_r[:, 0:512],
                                 start=True, stop=True)
                nc.tensor.matmul(st_ps[:, 512:NS], lhs, r_r[:, 512:NS],
                                 start=True, stop=True)
                pt = ptp.tile([P, NS], BF16, tag="pt", name=f"pt{j}")
                nc.scalar.activation(pt[:], st_ps[:], AF.Exp, bias=shift[:])
                pts.append(pt)
                if j >= 2:
                    consume(j - 2, pts[j - 2])
                if j >= 1:
                    rs_add(j - 1, pts[j - 1])
            rs_add(NB - 1, pts[NB - 1])

            # broadcast-fold both accumulators with an all-ones stationary,
            # summing them in PSUM: rb[p, n] = rowsum[n] on every partition.
            # Emitted BEFORE the final PV consumes: the ln/exp reciprocal
            # chain it feeds is longer than the consumes' slack (the h
            # multiply waits on the reciprocal, not the other way around).
            rb_ps = stp.tile([P, NS], F32, tag="st", name="rb_ps")
            nc.tensor.matmul(rb_ps[:, 0:512], onesM[:], rsA[:, 0:512],
                             start=True, stop=False)
            nc.tensor.matmul(rb_ps[:, 0:512], onesM[:], rsB[:, 0:512],
                             start=False, stop=True)
            nc.tensor.matmul(rb_ps[:, 512:NS], onesM[:], rsA[:, 512:NS],
                             start=True, stop=False)
            nc.tensor.matmul(rb_ps[:, 512:NS], onesM[:], rsB[:, 512:NS],
                             start=False, stop=True)
            for jj in (NB - 2, NB - 1):
                consume(jj, pts[jj])

            # 1/rowsum = exp(-ln(rowsum)): both in the pinned table set;
            # covers the whole fp32 range unlike the ACT reciprocal.
            lnr = pp.tile([P, NS], F32)
            nc.scalar.activation(lnr[:], rb_ps[:], AF.Ln)
            rbinv = pp.tile([P, NS], F32)
            nc.scalar.activation(rbinv[:], lnr[:], AF.Exp, scale=-1.0)

            # h = h_un / rowsum (fp16 for the Wo matmul), by halves so the
            # Wo matmul overlaps the second multiply
            nc.vector.tensor_mul(h_sb[:, 0:512], h_ps[:, 0:512],
                                 rbinv[:, 0:512])
            nc.vector.tensor_mul(h_sb[:, 512:NS], h_ps[:, 512:NS],
                                 rbinv[:, 512:NS])

        # ------------- output projection + residual + GroupNorm + SiLU ----
        with tc.tile_pool(name="ep", bufs=1, space="PSUM") as ep:
            a_ps = ep.tile([P, NS], F32, tag="a")
            nc.tensor.matmul(a_ps[:, 0:512], woT_b, h_sb[:, 0:512],
                             start=True, stop=True)
            nc.tensor.matmul(a_ps[:, 512:NS], woT_b, h_sb[:, 512:NS],
                             start=True, stop=True)
            y_sb = pp.tile([P, NS], F32)
            nc.vector.tensor_add(y_sb[:, 0:512], a_ps[:, 0:512],
                                 xs_sb[:, 0:512])
            nc.vector.tensor_add(y_sb[:, 512:NS], a_ps[:, 512:NS],
                                 xs_sb[:, 512:NS])

            # per-channel partial stats over the local 1024 columns; halves
            # so the first half's reductions overlap the second half's add
            hsum = pp.tile([P, 2], F32)
            nc.vector.reduce_sum(hsum[:, 0:1], y_sb[:, 0:512], axis=AX.X)
            sq_sb = pp.tile([P, NS], F32)
            nc.scalar.activation(sq_sb[:, 0:512], y_sb[:, 0:512], AF.Square,
                                 accum_out=hsum[:, 1:2])
            hsum2 = pp.tile([P, 2], F32)
            nc.vector.reduce_sum(hsum2[:, 0:1], y_sb[:, 512:NS], axis=AX.X)
            nc.scalar.activation(sq_sb[:, 512:NS], y_sb[:, 512:NS], AF.Square,
                                 accum_out=hsum2[:, 1:2])
            nc.vector.tensor_add(stat_sb[:], hsum[:], hsum2[:])

            # ONE 8-rank AllReduce on a batch-masked [128, 4] payload:
            # cols 0-1 carry this core's stats if it is a batch-0 core,
            # cols 2-3 if batch-1. Two concurrent 4-rank group collectives
            # serialize on the CC machinery (~15us extra for the second
            # group); a single 8-rank op avoids that. Each core selects its
            # batch's half post-reduce. Silu table set preloads in flight.
            ms_sb = pp.tile([P, 4], F32)
            nc.vector.tensor_scalar(out=ms_sb[:, 0:2], in0=stat_sb[:],
                                    scalar1=bsel0, scalar2=None,
                                    op0=ALU.mult)
            nc.vector.tensor_scalar(out=ms_sb[:, 2:4], in0=stat_sb[:],
                                    scalar1=bsel1, scalar2=None,
                                    op0=ALU.mult)
            d_st1 = dp.tile([P, 4], F32)
            d_st2 = dp.tile([P, 4], F32)
            nc.sync.dma_start(out=d_st1[:], in_=ms_sb[:])
            dumo = pp.tile([1, 1], F32)
            nc.scalar.activation(dumo[:], stat_sb[0:1, 0:1], AF.Silu)
            nc.gpsimd.collective_compute(
                "AllReduce", ALU.add,
                replica_groups=[[0, 1, 2, 3, 4, 5, 6, 7]],
                ins=[d_st1.opt()], outs=[d_st2.opt()],
            )
            ast_sb = pp.tile([P, 4], F32)
            nc.sync.dma_start(out=ast_sb[:], in_=d_st2[:])
            sel0 = pp.tile([P, 2], F32)
            nc.vector.tensor_scalar(out=sel0[:], in0=ast_sb[:, 0:2],
                                    scalar1=bsel0, scalar2=None,
                                    op0=ALU.mult)
            sel1 = pp.tile([P, 2], F32)
            nc.vector.tensor_scalar(out=sel1[:], in0=ast_sb[:, 2:4],
                                    scalar1=bsel1, scalar2=None,
                                    op0=ALU.mult)
            ast_c = pp.tile([P, 2], F32)
            nc.vector.tensor_add(ast_c[:], sel0[:], sel1[:])

            # fold channels -> groups; gsel carries 1/CNT so this yields
            # [mean, E[y^2]] per group directly
            gs_ps = ep.tile([NGROUPS, 2], F32, tag="gs")
            nc.tensor.matmul(gs_ps[:], gsel_c, ast_c[:], start=True, stop=True)
            mg = pp.tile([NGROUPS, 2], F32)
            nc.vector.tensor_copy(mg[:], gs_ps[:])
            msq = pp.tile([NGROUPS, 1], F32)
            nc.vector.tensor_mul(msq[:], mg[:, 0:1], mg[:, 0:1])
            var8 = pp.tile([NGROUPS, 1], F32)
            nc.vector.tensor_sub(var8[:], mg[:, 1:2], msq[:])
            # rstd = 1/sqrt(var + eps) via bit-trick + 2 Newton steps on the
            # DVE ([8,1] tiles) — keeps the pinned ACT table set intact.
            ve8 = pp.tile([NGROUPS, 1], F32)
            nc.vector.tensor_scalar_add(ve8[:], in0=var8[:], scalar1=EPS)
            I32 = mybir.dt.int32
            magic = pp.tile([NGROUPS, 1], I32)
            nc.vector.memset(magic[:], 0x5F3759DF)
            ish = pp.tile([NGROUPS, 1], I32)
            nc.vector.tensor_scalar(out=ish[:], in0=ve8.bitcast(I32),
                                    scalar1=1, scalar2=None,
                                    op0=ALU.arith_shift_right)
            y0i = pp.tile([NGROUPS, 1], I32)
            nc.vector.tensor_sub(y0i[:], magic[:], ish[:])
            ycur = y0i.bitcast(F32)
            for it in range(2):
                yy = pp.tile([NGROUPS, 1], F32, name=f"yy{it}")
                nc.vector.tensor_mul(yy[:], ycur[:], ycur[:])
                vy2 = pp.tile([NGROUPS, 1], F32, name=f"vy2{it}")
                nc.vector.tensor_mul(vy2[:], ve8[:], yy[:])
                hh = pp.tile([NGROUPS, 1], F32, name=f"hh{it}")
                nc.vector.tensor_scalar(out=hh[:], in0=vy2[:], scalar1=-0.5,
                                        scalar2=1.5, op0=ALU.mult, op1=ALU.add)
                ynew = pp.tile([NGROUPS, 1], F32, name=f"ynew{it}")
                nc.vector.tensor_mul(ynew[:], ycur[:], hh[:])
                ycur = ynew
            gval = pp.tile([NGROUPS, 2], F32)
            nc.vector.tensor_copy(gval[:, 0:1], mg[:, 0:1])
            nc.vector.tensor_copy(gval[:, 1:2], ycur[:])

            # broadcast group stats back to channels: [128, 2] = G @ gval
            pc_ps = ep.tile([P, 2], F32, tag="pc")
            nc.tensor.matmul(pc_ps[:], gselT_c[:], gval[:], start=True, stop=True)
            pc_sb = pp.tile([P, 2], F32)
            nc.vector.tensor_copy(pc_sb[:], pc_ps[:])

            # fuse (y - mean)*rstd*gamma + beta into one pass:
            # A = rstd*gamma, B = beta - mean*A, z = y*A + B
            A_sb = pp.tile([P, 1], F32)
            nc.vector.tensor_mul(A_sb[:], pc_sb[:, 1:2], gamma_sb)
            t_sb = pp.tile([P, 1], F32)
            nc.vector.tensor_mul(t_sb[:], pc_sb[:, 0:1], A_sb[:])
            B_sb = pp.tile([P, 1], F32)
            nc.vector.tensor_sub(B_sb[:], beta_sb, t_sb[:])
            z_sb = pp.tile([P, NS], F32)
            nc.vector.tensor_scalar(out=z_sb[:], in0=y_sb[:],
                                    scalar1=A_sb[:], scalar2=B_sb[:],
                                    op0=ALU.mult, op1=ALU.add)
            o_sb = pp.tile([P, NS], F32)
            nc.scalar.activation(o_sb[:], z_sb[:], AF.Silu)
            nc.sync.dma_start(out=out[:], in_=o_sb[:])


_NC_CACHE = None


def _get_nc():
    global _NC_CACHE
    if _NC_CACHE is None:
        _NC_CACHE = _build_nc()
    return _NC_CACHE


def make_in_maps(x, Wq, Wk, Wv, Wo, gamma, beta):
    x = np.asarray(x, dtype=np.float32)
    B, C = x.shape[0], x.shape[1]
    xf = np.ascontiguousarray(x.reshape(B, C, -1))
    xf16 = xf.astype(np.float16)
    Wq = np.asarray(Wq, dtype=np.float32)
    Wk = np.asarray(Wk, dtype=np.float32)
    WvT = np.asarray(Wv, dtype=np.float32).T
    WoT = np.asarray(Wo, dtype=np.float32).T
    g = np.asarray(gamma, dtype=np.float32).reshape(P, 1)
    b = np.asarray(beta, dtype=np.float32).reshape(P, 1)
    ident = np.eye(P, dtype=np.float32).astype(NPBF16)
    gs = np.zeros((P, NGROUPS), dtype=np.float32)
    gs[np.arange(P), np.arange(P) // (P // NGROUPS)] = 1.0
    gsT = np.ascontiguousarray(gs.T)
    pbm = np.ascontiguousarray(
        np.concatenate([Wq, Wk, WvT, WoT], axis=1)).astype(np.float16)
    assert pbm.shape == (P, NPB)
    in_maps = []
    for core in range(NCORES):
        bi, s = core // 4, core % 4
        bsel = np.zeros((P, 2), dtype=np.float32)
        bsel[:, bi] = 1.0
        pfm = np.ascontiguousarray(
            np.concatenate([gs * (1.0 / CNT), g, b, bsel],
                           axis=1)).astype(np.float32)
        assert pfm.shape == (P, NPF)
        in_maps.append({
            "xb": xf16[bi],
            "xs16": np.ascontiguousarray(xf16[bi][:, s * NS:(s + 1) * NS]),
            "xs": np.ascontiguousarray(xf[bi][:, s * NS:(s + 1) * NS]),
            "pb": pbm, "idb": ident, "pf": pfm, "gselT": gsT,
        })
    return in_maps


def assemble(results, spatial=(16, 16, 16)):
    y = np.empty((2, P, N), dtype=np.float32)
    for core in range(NCORES):
        bi, s = core // 4, core % 4
        y[bi][:, s * NS:(s + 1) * NS] = results[core]["out"]
    return y.reshape(2, P, *spatial)


def kernel(x, Wq, Wk, Wv, Wo, gamma, beta):
    nc = _get_nc()
    in_maps = make_in_maps(x, Wq, Wk, Wv, Wo, gamma, beta)
    res = run_bass_kernel_spmd(nc, in_maps, list(range(NCORES)))
    return assemble(res.results, spatial=tuple(np.asarray(x).shape[2:]))
